# revision 7
# baseline (speedup 1.0000x reference)
"""BoxTightnessPriorLoss Trainium2 kernel (v2: separable-marginal DoubleRow design).

Inputs (full, host-side):
  logits:    (2, 4, 128, 128, 128) float32   -- (B, C, W, H, D)
  box_masks: (2, 4, 4, 128, 128, 128) bool   -- (B, C, N, W, H, D), axis-aligned boxes

Math: every box mask is a product of three interval indicators
mask[n,w,h,d] = mw[n,w]*mh[n,h]*md[n,d], so each slice profile is a
bilinear contraction of logits with two marginals:
  T_d[n,d] = sum_{w,h} mw mh L      (then sl_d = md * T_d)
  U[n,h]   = sum_{w,d} mw md L      (then sl_h = mh * U)
  V[n,w]   = sum_{h,d} mh md L      (then sl_w = mw * V)
Marginals are computed exactly on host via stride-16 subsampling (box
sides >= 16 always contain a multiple of 16).

Sharding: one core per (b, c) pair (B*C = 8 cores). Per core the device
streams two fp8 layouts of logits[b,c] (2 MiB each):
  lg[w, hpos*128 + d]   and   lt[d, hpos*128 + w]
with h-planes permuted so that each 1024-col block q holds planes
h = 8q+2j+kt at position (kt, j) -- this makes every PE pass a clean 3D
[part, kt, f] DoubleRow fp8 access pattern:
  T: 16 matmuls over lg, stationary mw*mh  -> PSUM (16,512) j-diag blocks
  U: 16 matmuls over lt, stationary md*mw  -> PSUM (16,512)  (4D moving)
  V: 16 matmuls over lt, stationary md*mh  -> PSUM (16,512)
Host extracts the j-diagonal blocks and finishes the tiny per-(b,c,n)
segment/relu/square/sum math in float32.
"""
import os
import numpy as np

B, C, N, DM = 2, 4, 4, 128
SEG_W = 8
N_SEG = DM // SEG_W  # 16
N_CORES = 8
NQ = 16  # 1024-col blocks per pass

# plane position p = q*8 + kt*4 + j  holds  h = 8q + 2j + kt
_p = np.arange(DM)
HPERM = (8 * (_p >> 3) + 2 * (_p & 3) + ((_p >> 2) & 1)).astype(np.int64)

_compiled = None

# U-pass moving AP is 4D ([d][kt][j][h]); set to 0 to fall back to the
# 3D no-j-block variant (64 accumulating matmuls) if 4D fails.
U_4D = bool(int(os.environ.get("BOXLOSS_U4D", "1")))


def _install_wait_split_patch():
    """This container's walrus (CoreV3) allows only ONE sync-wait per
    instruction; TileContext can attach several.  Split any instruction
    carrying N>1 waits into N-1 preceding wait-only NoOps (same engine)."""
    import concourse.tile as _tile
    import concourse.mybir as _mybir

    if getattr(_tile.TileContext, "_ant_wait_split", False):
        return
    _orig = _tile.TileContext.schedule_and_allocate

    def _split_multi_waits(nc):
        for func in nc.m.functions:
            for bb in func.blocks:
                insts = bb.instructions
                i = 0
                while i < len(insts):
                    inst = insts[i]
                    si = getattr(inst, "sync_info", None)
                    if si is not None and si.on_wait and len(si.on_wait) > 1:
                        waits = list(si.on_wait)
                        si.on_wait = [waits[-1]]
                        nops = []
                        for w in waits[:-1]:
                            nop = _mybir.InstNoOp(
                                name=nc.get_next_instruction_name(),
                                engine=inst.engine,
                                sync_info=_mybir.SyncInfo(on_wait=[w], on_update=[]),
                                bass_nofuse=True,
                            )
                            nops.append(nop)
                            nc.register_instruction(nop, overwrite=True)
                        insts[i:i] = nops
                        i += len(nops)
                    i += 1

    def _patched(self, *a, **kw):
        ret = _orig(self, *a, **kw)
        _split_multi_waits(self.nc)
        return ret

    _tile.TileContext.schedule_and_allocate = _patched
    _tile.TileContext._ant_wait_split = True


def _build():
    import concourse.bass as bass
    import concourse.tile as tile
    from concourse import mybir

    _install_wait_split_patch()

    f32 = mybir.dt.float32
    bf16 = mybir.dt.bfloat16
    f8 = mybir.dt.float8e4
    DR = mybir.MatmulPerfMode.DoubleRow

    nc = bass.Bass()
    lg = nc.dram_tensor("lg", [DM, DM * DM], f8, kind="ExternalInput")
    lt = nc.dram_tensor("lt", [DM, DM * DM], f8, kind="ExternalInput")
    # g3 cols: [0:512] GT (w-part), [512:1024] GU (d-part), [1024:1536] GV (d-part)
    g3 = nc.dram_tensor("g3", [DM, 3 * 512], f8, kind="ExternalInput")
    o_all = nc.dram_tensor("o_all", [16, 3 * 512], f32, kind="ExternalOutput")

    with tile.TileContext(nc) as tc:
        with (
            tc.tile_pool(name="consts", bufs=1) as consts,
            tc.tile_pool(name="big", bufs=1) as big,
            tc.tile_pool(name="outs", bufs=1) as outs,
            tc.tile_pool(name="wpsum", bufs=2, space="PSUM") as wpsum,
            tc.tile_pool(name="apsum", bufs=1, space="PSUM") as apsum,
        ):
            warm = consts.tile([DM, 512], bf16)
            nc.vector.memset(warm[:], 0.0)

            G = big.tile([DM, 3 * 512], f8)
            Lg = big.tile([DM, DM * DM], f8)
            Lt = big.tile([DM, DM * DM], f8)

            # ---- input DMAs split across both HWDGE rings (sync + scalar),
            # per-ring FIFO: stationaries, then lt (V+U), then lg (T).
            nc.sync.dma_start(out=G[:], in_=g3[:])
            for lo, hi in [(0, 4096), (4096, 8192)]:
                nc.sync.dma_start(out=Lt[:, lo:hi], in_=lt[:, lo:hi])
            for lo, hi in [(8192, 12288), (12288, 16384)]:
                nc.scalar.dma_start(out=Lt[:, lo:hi], in_=lt[:, lo:hi])
            for lo, hi in [(0, 6144), (6144, 8192)]:
                nc.sync.dma_start(out=Lg[:, lo:hi], in_=lg[:, lo:hi])
            for lo, hi in [(8192, 14336), (14336, 16384)]:
                nc.scalar.dma_start(out=Lg[:, lo:hi], in_=lg[:, lo:hi])

            # ---- PE clock warm-up while DMAs stream
            for i in range(12):
                wp = wpsum.tile([1, 512], f32, tag="wp")
                nc.tensor.matmul(wp[:], warm[:, :1], warm[:], start=True, stop=True)

            p_t = apsum.tile([16, 512], f32)
            p_u = apsum.tile([16, 512], f32)
            p_v = apsum.tile([16, 512], f32)

            def dr3(pout, gcol, L, q, start, stop):
                nc.tensor.matmul(
                    pout,
                    G[:, gcol + 32 * q:gcol + 32 * (q + 1)].rearrange(
                        "p (kt m) -> p kt m", kt=2),
                    L[:, 1024 * q:1024 * (q + 1)].rearrange(
                        "p (kt f) -> p kt f", kt=2),
                    start=start, stop=stop, perf_mode=DR,
                )

            # ---- V pass (lt, pipelined on its chunks)
            for q in range(NQ):
                dr3(p_v[:], 1024, Lt, q, q == 0, q == NQ - 1)

            # ---- U pass (lt, needs full lt)
            # f-order (h, j): per h the feed reads the 8 contiguous bytes
            # w = 8q..8q+7 (kt, j interleaved), then strides 128 -- much
            # friendlier to the moving-AP walker than 2-byte runs.
            if U_4D:
                Lt_u = Lt[:].rearrange(
                    "p (h qq j kt) -> p qq kt h j", qq=NQ, j=4, kt=2)
                for q in range(NQ):
                    nc.tensor.matmul(
                        p_u[:],
                        G[:, 512 + 32 * q:512 + 32 * (q + 1)].rearrange(
                            "p (kt m) -> p kt m", kt=2),
                        Lt_u[:, q],
                        start=(q == 0), stop=(q == NQ - 1), perf_mode=DR,
                    )
            else:
                # 3D fallback: 64 matmuls, stationary [d, kt, 16] built from
                # the same GU data but addressed per w-pair s: w = 2s+kt.
                # GU cols are [q][kt][j][n]; s = 4q + j' groups don't match
                # that layout, so reuse gu2 region instead (host packs it).
                Lt_u3 = Lt[:].rearrange("p (h s kt) -> p s kt h", s=64, kt=2)
                for s in range(64):
                    nc.tensor.matmul(
                        p_u[:4, :DM],
                        G[:, 512 + 8 * s:512 + 8 * (s + 1)].rearrange(
                            "p (kt m) -> p kt m", kt=2),
                        Lt_u3[:, s],
                        start=(s == 0), stop=(s == 63), perf_mode=DR,
                    )

            # ---- T pass (lg, pipelined on its chunks)
            for q in range(NQ):
                dr3(p_t[:], 0, Lg, q, q == 0, q == NQ - 1)

            stage = outs.tile([16, 3 * 512], f32)
            nc.scalar.copy(stage[:, 1024:1536], p_v[:])
            if U_4D:
                nc.vector.tensor_copy(stage[:, 512:1024], p_u[:])
            else:
                nc.vector.memset(stage[:, 512:1024], 0.0)
                nc.vector.tensor_copy(stage[:4, 512:512 + DM], p_u[:4, :DM])
            # ship U+V as soon as they are staged; T separately at the end
            nc.sync.dma_start(out=o_all[:, 512:1536], in_=stage[:, 512:1536])
            nc.vector.tensor_copy(stage[:, 0:512], p_t[:])
            nc.sync.dma_start(out=o_all[:, 0:512], in_=stage[:, 0:512])

    return nc


def _host_marginals(box_masks):
    """Exact interval marginals via stride-16 subsampling (sides >= 16)."""
    mw = box_masks[:, :, :, :, ::16, ::16].any(axis=(4, 5))  # (B,C,N,W)
    mh = box_masks[:, :, :, ::16, :, ::16].any(axis=(3, 5))  # (B,C,N,H)
    md = box_masks[:, :, :, ::16, ::16, :].any(axis=(3, 4))  # (B,C,N,D)
    return mw, mh, md


def _build_g(a, b_sel):
    """G[part, q, kt, j, n] = a[n, part] * b_sel[n, q, kt, j] -> (128, 512)."""
    g = np.einsum('np,nqkj->pqkjn', a.astype(np.float32),
                  b_sel.astype(np.float32))
    return g.reshape(DM, 512)


def _diag_extract(o):
    """o[(4j+n), (128j+x)] diag blocks -> (N, 128) summed over j."""
    r = np.zeros((N, DM), np.float32)
    for j in range(4):
        r += o[4 * j:4 * j + 4, 128 * j:128 * (j + 1)]
    return r


def _extract_u(o):
    """U part of the device output -> (N, h) in natural h order."""
    if U_4D:
        # o_u[(4j+n), (4*hpos+j)] diag -> U_p[n, hpos]
        ou = o[:, 512:1024].reshape(4, N, DM, 4)   # [j, n, hpos, j']
        U_p = np.einsum('jnhj->nh', ou)
    else:
        U_p = o[:4, 512:512 + DM].copy()           # (n, hpos)
    U = np.zeros_like(U_p)
    U[:, HPERM] = U_p
    return U


def _finish_core(o, mw, mh, md):
    """Host finisher: o is the (16, 1536) device output for one (b,c)."""
    T_d = _diag_extract(o[:, 0:512])            # (n, d)
    U = _extract_u(o)
    V = _diag_extract(o[:, 1024:1536])          # (n, w)

    mwf = mw.astype(np.float32)
    mhf = mh.astype(np.float32)
    mdf = md.astype(np.float32)
    sl_d = T_d * mdf
    sl_h = U * mhf
    sl_w = V * mwf

    def axis_err(sl, mk):
        seg_vals = sl.reshape(N, N_SEG, SEG_W).sum(axis=2, dtype=np.float32)
        seg_cnt = mk.reshape(N, N_SEG, SEG_W).sum(axis=2)
        valid = seg_cnt > 0
        mean = seg_vals / np.where(valid, seg_cnt, 1).astype(np.float32)
        err = np.where(valid, np.maximum(np.float32(1.0) - mean, np.float32(0.0)),
                       np.float32(0.0))
        return err.sum(axis=1, dtype=np.float32)

    e_d = axis_err(sl_d, md)
    e_h = axis_err(sl_h, mh)
    e_w = axis_err(sl_w, mw)
    error = (e_d + e_h + e_w) * np.float32(SEG_W)
    error = np.where(error >= 0, np.square(error), np.float32(0.0))
    return error.sum(dtype=np.float32)


def _prep_core(L, mw, mh, md):
    """Per-(b,c) device inputs. L: (W,H,D) f32; marginals: (N, 128) bool."""
    import ml_dtypes
    f8 = ml_dtypes.float8_e4m3
    lg8 = np.ascontiguousarray(L[:, HPERM, :]).reshape(DM, DM * DM).astype(f8)
    ltr = np.ascontiguousarray(L.transpose(2, 1, 0)[:, HPERM, :])
    lt8 = ltr.reshape(DM, DM * DM).astype(f8)
    b_w = mw[:, HPERM].reshape(N, NQ, 2, 4)
    b_h = mh[:, HPERM].reshape(N, NQ, 2, 4)
    gt = _build_g(mw, b_h)        # [w | mh-sel]
    gv = _build_g(md, b_h)        # [d | mh-sel]
    if U_4D:
        gu = _build_g(md, b_w)    # [d | mw-sel]
    else:
        # gu2[d, s, kt, n] = md[n,d] * mw[n, 2s+kt]
        b_w2 = mw.reshape(N, 64, 2)
        gu = np.einsum('np,nsk->pskn', md.astype(np.float32),
                       b_w2.astype(np.float32)).reshape(DM, 512)
    g3 = np.concatenate([gt, gu, gv], axis=1).astype(f8)
    return {"lg": lg8, "lt": lt8, "g3": g3}


def kernel(logits: np.ndarray, box_masks: np.ndarray) -> np.ndarray:
    global _compiled
    from concourse.bass_utils import run_bass_kernel_spmd

    if _compiled is None:
        _compiled = _build()
    nc = _compiled

    logits = np.ascontiguousarray(logits, dtype=np.float32)
    mw, mh, md = _host_marginals(box_masks)

    in_maps = []
    for core in range(N_CORES):
        b, c = divmod(core, C)
        in_maps.append(_prep_core(logits[b, c], mw[b, c], mh[b, c], md[b, c]))

    trace = bool(int(os.environ.get("BOXLOSS_TRACE", "0")))
    res = run_bass_kernel_spmd(nc, in_maps, core_ids=list(range(N_CORES)), trace=trace)
    if trace:
        kernel._last_result = res

    total = np.float32(0.0)
    for core in range(N_CORES):
        b, c = divmod(core, C)
        total += _finish_core(np.asarray(res.results[core]["o_all"], np.float32),
                              mw[b, c], mh[b, c], md[b, c])
    return np.float32(total)


# revision 9
# speedup vs baseline: 1.2067x; 1.2067x over previous
"""BoxTightnessPriorLoss Trainium2 kernel (v2: separable-marginal DoubleRow design).

Inputs (full, host-side):
  logits:    (2, 4, 128, 128, 128) float32   -- (B, C, W, H, D)
  box_masks: (2, 4, 4, 128, 128, 128) bool   -- (B, C, N, W, H, D), axis-aligned boxes

Math: every box mask is a product of three interval indicators
mask[n,w,h,d] = mw[n,w]*mh[n,h]*md[n,d], so each slice profile is a
bilinear contraction of logits with two marginals:
  T_d[n,d] = sum_{w,h} mw mh L      (then sl_d = md * T_d)
  U[n,h]   = sum_{w,d} mw md L      (then sl_h = mh * U)
  V[n,w]   = sum_{h,d} mh md L      (then sl_w = mw * V)
Marginals are computed exactly on host via stride-16 subsampling (box
sides >= 16 always contain a multiple of 16).

Sharding: one core per (b, c) pair (B*C = 8 cores). Per core the device
streams two fp8 layouts of logits[b,c] (2 MiB each):
  lg[w, hpos*128 + d]   and   lt[d, hpos*128 + w]
with h-planes permuted so that each 1024-col block q holds planes
h = 8q+2j+kt at position (kt, j) -- this makes every PE pass a clean 3D
[part, kt, f] DoubleRow fp8 access pattern:
  T: 16 matmuls over lg, stationary mw*mh  -> PSUM (16,512) j-diag blocks
  U: 16 matmuls over lt, stationary md*mw  -> PSUM (16,512)  (4D moving)
  V: 16 matmuls over lt, stationary md*mh  -> PSUM (16,512)
Host extracts the j-diagonal blocks and finishes the tiny per-(b,c,n)
segment/relu/square/sum math in float32.
"""
import os
import numpy as np

B, C, N, DM = 2, 4, 4, 128
SEG_W = 8
N_SEG = DM // SEG_W  # 16
N_CORES = 8
NQ = 16  # 1024-col blocks per pass

# plane position p = q*8 + kt*4 + j  holds  h = 8q + 2j + kt
_p = np.arange(DM)
HPERM = (8 * (_p >> 3) + 2 * (_p & 3) + ((_p >> 2) & 1)).astype(np.int64)

_compiled = None

# U-pass moving AP is 4D ([d][kt][j][h]); set to 0 to fall back to the
# 3D no-j-block variant (64 accumulating matmuls) if 4D fails.
U_4D = bool(int(os.environ.get("BOXLOSS_U4D", "1")))


def _install_wait_split_patch():
    """This container's walrus (CoreV3) allows only ONE sync-wait per
    instruction; TileContext can attach several.  Split any instruction
    carrying N>1 waits into N-1 preceding wait-only NoOps (same engine)."""
    import concourse.tile as _tile
    import concourse.mybir as _mybir

    if getattr(_tile.TileContext, "_ant_wait_split", False):
        return
    _orig = _tile.TileContext.schedule_and_allocate

    def _split_multi_waits(nc):
        for func in nc.m.functions:
            for bb in func.blocks:
                insts = bb.instructions
                i = 0
                while i < len(insts):
                    inst = insts[i]
                    si = getattr(inst, "sync_info", None)
                    if si is not None and si.on_wait and len(si.on_wait) > 1:
                        waits = list(si.on_wait)
                        si.on_wait = [waits[-1]]
                        nops = []
                        for w in waits[:-1]:
                            nop = _mybir.InstNoOp(
                                name=nc.get_next_instruction_name(),
                                engine=inst.engine,
                                sync_info=_mybir.SyncInfo(on_wait=[w], on_update=[]),
                                bass_nofuse=True,
                            )
                            nops.append(nop)
                            nc.register_instruction(nop, overwrite=True)
                        insts[i:i] = nops
                        i += len(nops)
                    i += 1

    def _patched(self, *a, **kw):
        ret = _orig(self, *a, **kw)
        _split_multi_waits(self.nc)
        return ret

    _tile.TileContext.schedule_and_allocate = _patched
    _tile.TileContext._ant_wait_split = True


def _build():
    import concourse.bass as bass
    import concourse.tile as tile
    from concourse import mybir

    _install_wait_split_patch()

    f32 = mybir.dt.float32
    bf16 = mybir.dt.bfloat16
    f8 = mybir.dt.float8e4
    DR = mybir.MatmulPerfMode.DoubleRow

    nc = bass.Bass()
    lg = nc.dram_tensor("lg", [DM, DM * DM], f8, kind="ExternalInput")
    lt = nc.dram_tensor("lt", [DM, DM * DM], f8, kind="ExternalInput")
    # g3 cols: [0:512] GT (w-part), [512:1024] GU (d-part), [1024:1536] GV (d-part)
    g3 = nc.dram_tensor("g3", [DM, 3 * 512], f8, kind="ExternalInput")
    o_all = nc.dram_tensor("o_all", [16, 3 * 512], f32, kind="ExternalOutput")

    with tile.TileContext(nc) as tc:
        with (
            tc.tile_pool(name="consts", bufs=1) as consts,
            tc.tile_pool(name="big", bufs=1) as big,
            tc.tile_pool(name="outs", bufs=1) as outs,
            tc.tile_pool(name="wpsum", bufs=2, space="PSUM") as wpsum,
            tc.tile_pool(name="apsum", bufs=1, space="PSUM") as apsum,
        ):
            warm = consts.tile([DM, 512], bf16)
            nc.vector.memset(warm[:], 0.0)

            G = big.tile([DM, 3 * 512], f8)
            Lg = big.tile([DM, DM * DM], f8)
            Lt = big.tile([DM, DM * DM], f8)

            # ---- input DMAs, one HWDGE ring (sync), FIFO order:
            # stationaries, then lt (V+U), then lg (T); tiny final lg chunk
            # so the T tail has data as early as possible.
            nc.sync.dma_start(out=G[:], in_=g3[:])
            for lo, hi in [(0, 6144), (6144, 12288), (12288, 16384)]:
                nc.sync.dma_start(out=Lt[:, lo:hi], in_=lt[:, lo:hi])
            for lo, hi in [(0, 6144), (6144, 12288), (12288, 15360),
                           (15360, 16384)]:
                nc.sync.dma_start(out=Lg[:, lo:hi], in_=lg[:, lo:hi])

            # ---- PE clock warm-up while DMAs stream
            for i in range(12):
                wp = wpsum.tile([1, 512], f32, tag="wp")
                nc.tensor.matmul(wp[:], warm[:, :1], warm[:], start=True, stop=True)

            p_t = apsum.tile([16, 512], f32)
            p_u = apsum.tile([16, 512], f32)
            p_v = apsum.tile([16, 512], f32)

            def dr3(pout, gcol, L, q, start, stop):
                nc.tensor.matmul(
                    pout,
                    G[:, gcol + 32 * q:gcol + 32 * (q + 1)].rearrange(
                        "p (kt m) -> p kt m", kt=2),
                    L[:, 1024 * q:1024 * (q + 1)].rearrange(
                        "p (kt f) -> p kt f", kt=2),
                    start=start, stop=stop, perf_mode=DR,
                )

            # ---- V pass (lt, pipelined on its chunks)
            for q in range(NQ):
                dr3(p_v[:], 1024, Lt, q, q == 0, q == NQ - 1)

            # ---- U pass (lt, needs full lt)
            # f-order (h, j): per h the feed reads the 8 contiguous bytes
            # w = 8q..8q+7 (kt, j interleaved), then strides 128 -- much
            # friendlier to the moving-AP walker than 2-byte runs.
            if U_4D:
                Lt_u = Lt[:].rearrange(
                    "p (h qq j kt) -> p qq kt h j", qq=NQ, j=4, kt=2)
                for q in range(NQ):
                    nc.tensor.matmul(
                        p_u[:],
                        G[:, 512 + 32 * q:512 + 32 * (q + 1)].rearrange(
                            "p (kt m) -> p kt m", kt=2),
                        Lt_u[:, q],
                        start=(q == 0), stop=(q == NQ - 1), perf_mode=DR,
                    )
            else:
                # 3D fallback: 64 matmuls, stationary [d, kt, 16] built from
                # the same GU data but addressed per w-pair s: w = 2s+kt.
                # GU cols are [q][kt][j][n]; s = 4q + j' groups don't match
                # that layout, so reuse gu2 region instead (host packs it).
                Lt_u3 = Lt[:].rearrange("p (h s kt) -> p s kt h", s=64, kt=2)
                for s in range(64):
                    nc.tensor.matmul(
                        p_u[:4, :DM],
                        G[:, 512 + 8 * s:512 + 8 * (s + 1)].rearrange(
                            "p (kt m) -> p kt m", kt=2),
                        Lt_u3[:, s],
                        start=(s == 0), stop=(s == 63), perf_mode=DR,
                    )

            # ---- T pass (lg, pipelined on its chunks)
            for q in range(NQ):
                dr3(p_t[:], 0, Lg, q, q == 0, q == NQ - 1)

            stage = outs.tile([16, 3 * 512], f32)
            nc.scalar.copy(stage[:, 1024:1536], p_v[:])
            if U_4D:
                nc.vector.tensor_copy(stage[:, 512:1024], p_u[:])
            else:
                nc.vector.memset(stage[:, 512:1024], 0.0)
                nc.vector.tensor_copy(stage[:4, 512:512 + DM], p_u[:4, :DM])
            # ship U+V as soon as they are staged; T separately at the end
            nc.sync.dma_start(out=o_all[:, 512:1536], in_=stage[:, 512:1536])
            nc.vector.tensor_copy(stage[:, 0:256], p_t[:, 0:256])
            nc.scalar.copy(stage[:, 256:512], p_t[:, 256:512])
            nc.sync.dma_start(out=o_all[:, 0:512], in_=stage[:, 0:512])

    return nc


def _host_marginals(box_masks):
    """Exact interval marginals via stride-16 subsampling (sides >= 16)."""
    mw = box_masks[:, :, :, :, ::16, ::16].any(axis=(4, 5))  # (B,C,N,W)
    mh = box_masks[:, :, :, ::16, :, ::16].any(axis=(3, 5))  # (B,C,N,H)
    md = box_masks[:, :, :, ::16, ::16, :].any(axis=(3, 4))  # (B,C,N,D)
    return mw, mh, md


def _build_g(a, b_sel):
    """G[part, q, kt, j, n] = a[n, part] * b_sel[n, q, kt, j] -> (128, 512)."""
    g = np.einsum('np,nqkj->pqkjn', a.astype(np.float32),
                  b_sel.astype(np.float32))
    return g.reshape(DM, 512)


def _diag_extract(o):
    """o[(4j+n), (128j+x)] diag blocks -> (N, 128) summed over j."""
    r = np.zeros((N, DM), np.float32)
    for j in range(4):
        r += o[4 * j:4 * j + 4, 128 * j:128 * (j + 1)]
    return r


def _extract_u(o):
    """U part of the device output -> (N, h) in natural h order."""
    if U_4D:
        # o_u[(4j+n), (4*hpos+j)] diag -> U_p[n, hpos]
        ou = o[:, 512:1024].reshape(4, N, DM, 4)   # [j, n, hpos, j']
        U_p = np.einsum('jnhj->nh', ou)
    else:
        U_p = o[:4, 512:512 + DM].copy()           # (n, hpos)
    U = np.zeros_like(U_p)
    U[:, HPERM] = U_p
    return U


def _finish_core(o, mw, mh, md):
    """Host finisher: o is the (16, 1536) device output for one (b,c)."""
    T_d = _diag_extract(o[:, 0:512])            # (n, d)
    U = _extract_u(o)
    V = _diag_extract(o[:, 1024:1536])          # (n, w)

    mwf = mw.astype(np.float32)
    mhf = mh.astype(np.float32)
    mdf = md.astype(np.float32)
    sl_d = T_d * mdf
    sl_h = U * mhf
    sl_w = V * mwf

    def axis_err(sl, mk):
        seg_vals = sl.reshape(N, N_SEG, SEG_W).sum(axis=2, dtype=np.float32)
        seg_cnt = mk.reshape(N, N_SEG, SEG_W).sum(axis=2)
        valid = seg_cnt > 0
        mean = seg_vals / np.where(valid, seg_cnt, 1).astype(np.float32)
        err = np.where(valid, np.maximum(np.float32(1.0) - mean, np.float32(0.0)),
                       np.float32(0.0))
        return err.sum(axis=1, dtype=np.float32)

    e_d = axis_err(sl_d, md)
    e_h = axis_err(sl_h, mh)
    e_w = axis_err(sl_w, mw)
    error = (e_d + e_h + e_w) * np.float32(SEG_W)
    error = np.where(error >= 0, np.square(error), np.float32(0.0))
    return error.sum(dtype=np.float32)


def _prep_core(L, mw, mh, md):
    """Per-(b,c) device inputs. L: (W,H,D) f32; marginals: (N, 128) bool."""
    import ml_dtypes
    f8 = ml_dtypes.float8_e4m3
    lg8 = np.ascontiguousarray(L[:, HPERM, :]).reshape(DM, DM * DM).astype(f8)
    ltr = np.ascontiguousarray(L.transpose(2, 1, 0)[:, HPERM, :])
    lt8 = ltr.reshape(DM, DM * DM).astype(f8)
    b_w = mw[:, HPERM].reshape(N, NQ, 2, 4)
    b_h = mh[:, HPERM].reshape(N, NQ, 2, 4)
    gt = _build_g(mw, b_h)        # [w | mh-sel]
    gv = _build_g(md, b_h)        # [d | mh-sel]
    if U_4D:
        gu = _build_g(md, b_w)    # [d | mw-sel]
    else:
        # gu2[d, s, kt, n] = md[n,d] * mw[n, 2s+kt]
        b_w2 = mw.reshape(N, 64, 2)
        gu = np.einsum('np,nsk->pskn', md.astype(np.float32),
                       b_w2.astype(np.float32)).reshape(DM, 512)
    g3 = np.concatenate([gt, gu, gv], axis=1).astype(f8)
    return {"lg": lg8, "lt": lt8, "g3": g3}


def kernel(logits: np.ndarray, box_masks: np.ndarray) -> np.ndarray:
    global _compiled
    from concourse.bass_utils import run_bass_kernel_spmd

    if _compiled is None:
        _compiled = _build()
    nc = _compiled

    logits = np.ascontiguousarray(logits, dtype=np.float32)
    mw, mh, md = _host_marginals(box_masks)

    in_maps = []
    for core in range(N_CORES):
        b, c = divmod(core, C)
        in_maps.append(_prep_core(logits[b, c], mw[b, c], mh[b, c], md[b, c]))

    trace = bool(int(os.environ.get("BOXLOSS_TRACE", "0")))
    res = run_bass_kernel_spmd(nc, in_maps, core_ids=list(range(N_CORES)), trace=trace)
    if trace:
        kernel._last_result = res

    total = np.float32(0.0)
    for core in range(N_CORES):
        b, c = divmod(core, C)
        total += _finish_core(np.asarray(res.results[core]["o_all"], np.float32),
                              mw[b, c], mh[b, c], md[b, c])
    return np.float32(total)
